# revision 15
# baseline (speedup 1.0000x reference)
"""Trainium2 Bass kernel for nn_BoundaryDiceLoss_82171314307268 (v2).

Sharding: pure data-parallel over 8 cores; core c handles sample c//2,
D-half c%2. Each core gets a [H=128(partitions), 70 D-slots, 130 w]
bf16 slab (64 owned D slices + 3 halo each side, D edge-replicated;
w cols 0/129 edge-replicated so neighbor reads at the w boundary see
the edge value) of out0, out1, and tgt7 = 7*target+1 in {1,8}.

Algorithm (v = (out1-out0 > 0) + tgt7 in {1,2,8,9}; multiplier 7 > 6
makes the 6-neighbor sum carry-free: sum == 6*center iff all 6 equal):
  E   = |c_v - 6 v|            boundary of either mask  (PE + ACT Abs)
  r   = conv3d(E, ball radius 2)   via 6 PE terms per 4-slot chunk:
        T5@E + T3@s3z + T3@F[w-1] + T3@F[w+1] + I@c4a + I@c4b
        s3z = E[z-1]+E[z+1], F = E+s3z   (GpSimd adds)
        c4a = E[w-2]+E[w+2], c4b = E[z-2]+E[z+2]  (DVE adds)
  m   = (r > 0.5)  exact {0,1} mask (r is an exact integer in PSUM)
  sums: S_m, S_pm = sum m*probs, S_mt = sum m*tgt7, S_mpt = sum mp*tgt7
        -> elementwise on DVE, column-reduced on PE with four
        one-hot-column stationary mats accumulating into one PSUM bank.
  host: S_tm = (S_mt-S_m)/7, S_ptm = (S_mpt-S_pm)/7, dice math.

All phases are interleaved chunk-wise in emission order so DMA / DVE /
PE / ACT / GpSimd overlap (the v1 kernel serialized phases; DVE sat at
80% busy with f32 1x-mode ops and 10us 1x-mode accumulation passes).
"""
import sys

sys.path.insert(0, "/opt/trn_rl_repo")

import numpy as np
import ml_dtypes

import concourse.bass as bass
import concourse.bacc as bacc
import concourse.tile as tile
import concourse.mybir as mybir
from concourse.bass_utils import run_bass_kernel_spmd

f32 = mybir.dt.float32
bf16 = mybir.dt.bfloat16
Alu = mybir.AluOpType
Act = mybir.ActivationFunctionType

P = 128
W = 128
WI = 130                     # input/v w-cols: data at [1,129), edges replicated
WE = 132                     # E w-cols: data at [2,130), zero pads outside
OWN = 64
HALO = 3
DEXT = OWN + 2 * HALO        # 70
B = 4
EPS = 1e-05

PIECE = 8                    # DMA / DVE / ACT batching (slots)
NPA = (DEXT + PIECE - 1) // PIECE     # 9 A-pieces (last has 6 slots)
CH = 4                       # PE chunk (512 moving cols)
NE = 17                      # E chunks, slots [1,69)
ND = 16                      # dilation chunks, owned slots [3,67)
NPQ = 8                      # product pieces (8 owned slots each)
NS16 = 4                     # s3z/f3 16-slot pieces


def _band(offsets, rep_edges=False):
    m = np.zeros((P, P), np.float32)
    for o in offsets:
        for i in range(P):
            j = i + o
            if 0 <= j < P:
                m[j, i] += 1.0
            elif rep_edges:
                m[min(max(j, 0), P - 1), i] += 1.0
    return m


def _const_mats():
    a1 = _band([-1, 1], rep_edges=True)
    mats = {"m_b": a1 - 6.0 * np.eye(P, dtype=np.float32),
            "m_t3": _band([-1, 0, 1]),
            "m_t5": _band([-2, -1, 0, 1, 2]),
            "m_id": np.eye(P, dtype=np.float32)}
    # column-reduce stationary: slice [:, 32a:32a+32] has ones only in
    # its local col 0, so the 4 slab-reductions run as concurrent 32-col
    # PE tiles writing PSUM rows 0/32/64/96
    w = np.zeros((P, P), np.float32)
    for a in range(4):
        w[:, 32 * a] = 1.0
    mats["w_red"] = w
    return mats


def _build_program():
    nc = bacc.Bacc("TRN2", target_bir_lowering=False, debug=False,
                   num_devices=8)
    d_out0 = nc.dram_tensor("out0", [P, DEXT * WI], bf16, kind="ExternalInput")
    d_out1 = nc.dram_tensor("out1", [P, DEXT * WI], bf16, kind="ExternalInput")
    d_tgt = nc.dram_tensor("tgt7", [P, DEXT * WI], bf16, kind="ExternalInput")
    d_mats = {n: nc.dram_tensor(n, [P, P], bf16, kind="ExternalInput")
              for n in ("m_b", "m_t3", "m_t5", "m_id", "w_red")}
    d_psums = nc.dram_tensor("psums", [P, 512], f32, kind="ExternalOutput")

    with tile.TileContext(nc) as tc:
        with tc.tile_pool(name="consts", bufs=1) as cp, \
             tc.tile_pool(name="slabs", bufs=1) as sp, \
             tc.tile_pool(name="inc", bufs=3) as kp, \
             tc.tile_pool(name="dilpre", bufs=2) as dp, \
             tc.tile_pool(name="prod", bufs=3) as qp, \
             tc.tile_pool(name="ps_e", bufs=3, space="PSUM") as ps_e, \
             tc.tile_pool(name="ps_p", bufs=3, space="PSUM") as ps_p, \
             tc.tile_pool(name="ps_r", bufs=1, space="PSUM") as ps_r:

            mats = {}

            def slab(name_, slots, cols, pool=sp, tag=None):
                t = pool.tile([P, slots * cols], bf16, tag=tag or name_,
                              name=name_)
                return t.rearrange("p (s w) -> p s w", w=cols)

            v3 = slab("v", DEXT, WI)
            e3 = slab("e", DEXT, WE)
            probs = slab("probs", OWN, W)
            tgt7 = slab("tgt7", DEXT, WI)
            rb = slab("rb", OWN, W)

            # bucketed emission: bucket index -> list of thunks
            buckets = {}

            def emit(bk, fn):
                buckets.setdefault(bk, []).append(fn)

            # ---- constants + E pad memsets ----
            # (emitted at bucket 0.5, after piece-0's data DMA dispatches:
            #  the first E matmul needs piece-0 data + m_b, everything else
            #  can trail; w_r* reduce mats are only needed at bucket 7+)
            def init():
                for n in ("m_b", "m_t3", "m_t5", "m_id"):
                    mats[n] = cp.tile([P, P], bf16, tag=n, name=n)
                    nc.scalar.dma_start(mats[n][:], d_mats[n][:])
                nc.vector.memset(e3[:, :, 0:2], 0.0)
                nc.vector.memset(e3[:, :, 130:132], 0.0)
            emit(0.5, init)

            def init2():
                mats["w_red"] = cp.tile([P, P], bf16, tag="w_red",
                                        name="w_red")
                nc.sync.dma_start(mats["w_red"][:], d_mats["w_red"][:])
            emit(3, init2)

            # ---- phase A ----
            in_tiles = {}
            for k in range(NPA):
                s0 = k * PIECE
                ns = min(PIECE, DEXT - s0)
                nf = ns * WI

                def dma_k(k=k, s0=s0, nf=nf, ns=ns):
                    c0 = kp.tile([P, PIECE * WI], bf16, tag="in0")
                    c1 = kp.tile([P, PIECE * WI], bf16, tag="in1")
                    # The head is DMA-dispatch-rate-bound (~620ns per
                    # dma_start, serial per engine queue): spread early
                    # pieces' dispatches across otherwise-idle engine
                    # queues so piece 0 lands ASAP.
                    eng1 = nc.scalar if k <= 2 else nc.sync
                    nc.sync.dma_start(c0[:, :nf],
                                      d_out0[:, s0 * WI:s0 * WI + nf])
                    eng1.dma_start(c1[:, :nf],
                                   d_out1[:, s0 * WI:s0 * WI + nf])
                    nc.gpsimd.dma_start(tgt7[:, s0:s0 + ns, :],
                                        d_tgt[:, s0 * WI:s0 * WI + nf])
                    in_tiles[k] = (c0, c1)
                emit(k, dma_k)

                def comp_k(k=k, s0=s0, ns=ns, nf=nf):
                    c0, c1 = in_tiles[k]
                    cd = kp.tile([P, PIECE * WI], bf16, tag="diff")
                    pr = kp.tile([P, PIECE * WI], bf16, tag="pred")
                    nc.vector.tensor_sub(cd[:, :nf], c1[:, :nf], c0[:, :nf])
                    # STT only has a 1x uop; TS(4x) + TT(2x) is faster
                    nc.vector.tensor_scalar(pr[:, :nf], cd[:, :nf], 0.0, None,
                                            op0=Alu.is_gt, op1=Alu.bypass)
                    cd3 = cd[:].rearrange("p (s w) -> p s w", w=WI)
                    pr3 = pr[:].rearrange("p (s w) -> p s w", w=WI)
                    nc.vector.tensor_add(v3[:, s0:s0 + ns, :], pr3[:, :ns, :],
                                         tgt7[:, s0:s0 + ns, :])
                    o0, o1 = max(s0, HALO), min(s0 + ns, HALO + OWN)
                    if o0 < o1:
                        nc.scalar.activation(
                            probs[:, o0 - HALO:o1 - HALO, :],
                            cd3[:, o0 - s0:o1 - s0, 1:129], Act.Sigmoid)
                emit(k + 1, comp_k)

            # ---- E phase: chunk g covers slots [1+4g, 5+4g) ----
            for g in range(NE):
                se = 1 + g * CH
                bk = (4 * g + 5) // PIECE + 2

                def e_g(g=g, se=se):
                    pe = ps_e.tile([P, CH * W], f32, tag="pe")
                    pe3 = pe[:].rearrange("p (s w) -> p s w", w=W)
                    sl = slice(se, se + CH)
                    nc.tensor.matmul(pe3[:], mats["m_b"][:], v3[:, sl, 1:129],
                                     start=True, stop=False)
                    nc.tensor.matmul(pe3[:], mats["m_id"][:], v3[:, sl, 0:128],
                                     start=False, stop=False)
                    nc.tensor.matmul(pe3[:], mats["m_id"][:], v3[:, sl, 2:130],
                                     start=False, stop=False)
                    nc.tensor.matmul(pe3[:], mats["m_id"][:],
                                     v3[:, se - 1:se - 1 + CH, 1:129],
                                     start=False, stop=False)
                    nc.tensor.matmul(pe3[:], mats["m_id"][:],
                                     v3[:, se + 1:se + 1 + CH, 1:129],
                                     start=False, stop=True)
                    nc.scalar.activation(e3[:, sl, 2:130], pe3[:], Act.Abs)
                emit(bk, e_g)

            # ---- dilation pre-sums (16-slot pieces; DVE — GpSimd tensor
            # ops throttle concurrent DVE via the shared port mux) ----
            s3z_t, f3_t, c4b_t = [None] * NS16, [None] * NS16, [None] * NS16
            for q in range(NS16):
                s0 = 3 + 16 * q
                bk = 2 * q + 5

                def pre16(q=q, s0=s0):
                    s3 = slab(f"s3z{q}", 16, WE, pool=dp, tag="s3z")
                    f3 = slab(f"f3{q}", 16, WE, pool=dp, tag="f3")
                    cb = slab(f"c4b{q}", 16, W, pool=dp, tag="c4b")
                    nc.vector.tensor_add(s3[:, :, :], e3[:, s0 - 1:s0 + 15, :],
                                         e3[:, s0 + 1:s0 + 17, :])
                    nc.vector.tensor_add(f3[:, :, :], e3[:, s0:s0 + 16, :],
                                         s3[:, :, :])
                    nc.vector.tensor_add(cb[:, :, :],
                                         e3[:, s0 - 2:s0 + 14, 2:130],
                                         e3[:, s0 + 2:s0 + 18, 2:130])
                    s3z_t[q], f3_t[q], c4b_t[q] = s3, f3, cb
                emit(bk, pre16)

            # ---- dilation matmuls + r copy: chunk j, slots [3+4j, 7+4j) ----
            for j in range(ND):
                s0 = 3 + 4 * j
                q16 = j // 4
                bk = 2 * q16 + 6

                def dil_j(j=j, s0=s0, q16=q16):
                    pp = ps_p.tile([P, CH * W], f32, tag="pp")
                    pp3 = pp[:].rearrange("p (s w) -> p s w", w=W)
                    sl = slice(s0, s0 + CH)
                    i16 = slice(4 * (j - 4 * q16), 4 * (j - 4 * q16) + 4)
                    s3, f3 = s3z_t[q16], f3_t[q16]
                    nc.tensor.matmul(pp3[:], mats["m_t5"][:], e3[:, sl, 2:130],
                                     start=True, stop=False)
                    nc.tensor.matmul(pp3[:], mats["m_t3"][:], s3[:, i16, 2:130],
                                     start=False, stop=False)
                    nc.tensor.matmul(pp3[:], mats["m_t3"][:], f3[:, i16, 1:129],
                                     start=False, stop=False)
                    nc.tensor.matmul(pp3[:], mats["m_t3"][:], f3[:, i16, 3:131],
                                     start=False, stop=False)
                    nc.tensor.matmul(pp3[:], mats["m_id"][:], e3[:, sl, 0:128],
                                     start=False, stop=False)
                    nc.tensor.matmul(pp3[:], mats["m_id"][:], e3[:, sl, 4:132],
                                     start=False, stop=False)
                    nc.tensor.matmul(pp3[:], mats["m_id"][:],
                                     c4b_t[q16][:, i16, :],
                                     start=False, stop=True)
                    nc.scalar.copy(rb[:, s0 - HALO:s0 - HALO + CH, :], pp3[:])
                emit(bk, dil_j)

            # ---- products (16-slot pieces) + PE column-reduce ----
            R = [None]
            for q in range(NS16):
                o0 = 16 * q                    # owned-slot offset
                bk = 2 * q + 7

                def prod_q(q=q, o0=o0):
                    m = qp.tile([P, 16 * W], bf16, tag="m")
                    mp = qp.tile([P, 16 * W], bf16, tag="mp")
                    mt = qp.tile([P, 16 * W], bf16, tag="mt")
                    mpt = qp.tile([P, 16 * W], bf16, tag="mpt")
                    m3 = m[:].rearrange("p (s w) -> p s w", w=W)
                    sl = slice(o0, o0 + 16)
                    tg = tgt7[:, HALO + o0:HALO + o0 + 16, 1:129]
                    nc.vector.tensor_scalar(m3[:], rb[:, sl, :], 0.5, None,
                                            op0=Alu.is_gt, op1=Alu.bypass)
                    mp3 = mp[:].rearrange("p (s w) -> p s w", w=W)
                    mt3 = mt[:].rearrange("p (s w) -> p s w", w=W)
                    mpt3 = mpt[:].rearrange("p (s w) -> p s w", w=W)
                    nc.vector.tensor_mul(mp3[:], m3[:], probs[:, sl, :])
                    nc.vector.tensor_mul(mt3[:], m3[:], tg)
                    nc.vector.tensor_mul(mpt3[:], mp3[:], tg)
                    if R[0] is None:
                        R[0] = ps_r.tile([P, 512], f32, tag="red",
                                         name="red")
                    slabs4 = (m, mp, mt, mpt)
                    for c in range(4):
                        for a in range(4):
                            nc.tensor.matmul(
                                R[0][32 * a:32 * a + 32, :],
                                mats["w_red"][:, 32 * a:32 * a + 32],
                                slabs4[a][:, c * 512:(c + 1) * 512],
                                start=(q == 0 and c == 0),
                                stop=(q == NS16 - 1 and c == 3),
                                tile_position=(0, 32 * a))
                emit(bk, prod_q)

            def fin():
                acc = sp.tile([P, 512], f32, tag="acc", name="acc")
                nc.scalar.copy(acc[:, :], R[0][:, :])
                nc.sync.dma_start(d_psums[:], acc[:, :])
            emit(99, fin)

            for bk in sorted(buckets):
                for fn in buckets[bk]:
                    fn()

    nc.compile()
    return nc


_CACHE = {}
TRACE = False
_LAST = {"exec_time_ns": None, "results": None}


def _get_program():
    if "nc" not in _CACHE:
        _CACHE["nc"] = _build_program()
    return _CACHE["nc"]


def last_exec_time_ns():
    return _LAST["exec_time_ns"]


def _core_slabs(output, target, c):
    s, h = c // 2, c % 2
    d0 = 0 if h == 0 else OWN
    sl = slice(d0, d0 + DEXT)
    out_p = np.pad(output[s], ((0, 0), (HALO, HALO), (0, 0), (0, 0)),
                   mode="edge")
    tgt_p = np.pad(target[s, 0], ((HALO, HALO), (0, 0), (0, 0)), mode="edge")

    def tr(a):  # [DEXT,H,W] -> [H, DEXT, W+2 rep] -> [H, DEXT*WI] bf16
        a = a.transpose(1, 0, 2)                       # [H, DEXT, W]
        a = np.pad(a, ((0, 0), (0, 0), (1, 1)), mode="edge")
        return np.ascontiguousarray(a).reshape(P, DEXT * WI).astype(
            ml_dtypes.bfloat16)

    return {"out0": tr(out_p[0][sl]), "out1": tr(out_p[1][sl]),
            "tgt7": tr(7.0 * tgt_p[sl] + 1.0)}


def kernel(output, target):
    output = np.asarray(output, dtype=np.float32)
    target = np.asarray(target, dtype=np.float32)
    nc = _get_program()

    mats = {n: m.astype(ml_dtypes.bfloat16) for n, m in _const_mats().items()}
    in_maps = []
    for c in range(8):
        m = _core_slabs(output, target, c)
        m.update(mats)
        in_maps.append(m)

    res = run_bass_kernel_spmd(nc, in_maps, list(range(8)), trace=TRACE)
    _LAST["exec_time_ns"] = res.exec_time_ns
    _LAST["results"] = res
    parts = np.zeros((B, 4), np.float64)
    for c in range(8):
        ps = res.results[c]["psums"].astype(np.float64)
        parts[c // 2] += ps[[0, 32, 64, 96], :].sum(axis=1)
    s_m, s_pm, s_mt, s_mpt = parts.T
    s_tm = (s_mt - s_m) / 7.0
    s_ptm = (s_mpt - s_pm) / 7.0
    dice = (2.0 * s_ptm + EPS) / (s_pm + s_tm + EPS)
    per_sample = np.where(s_m > 0, 1.0 - dice, 0.0)
    return np.float32(per_sample.sum() / B)
